# revision 1
# baseline (speedup 1.0000x reference)
"""Trainium2 Bass kernel for nn_CrossAtt (dual cross-attention + 3x3 conv + BN + ReLU).

Sharding: 8 cores = (sample s in 0..3) x (h-half in 0..1). Each core computes
its 32 output rows plus a 1-row attention halo on each side (34 rows = 2176
query positions, host-zero-padded so the program is SPMD-uniform), then runs
the 3x3 conv locally. No collectives.

Device layout choices:
- scoresT [m, n] comes straight off the PE (lhsT=k zero-padded to K=128,
  rhs=q), so softmax needs no transpose of the 4096x2176 matrix.
- exp on ScalarE (no max subtraction; |scores| <~ 5 so fp32 exp is safe).
- AV: out^T[n, 257] = expT.T @ [vT | ones]; col 256 accumulates the softmax
  denominator S for free.
- normalize by (gamma * mask / S) as a per-partition scalar; mask zeroes the
  fake padded query rows. PE-transposes the small [n,256] result to [256,n].
- residual + gamma*bv bias are folded into the host-prepared x?r inputs.
- conv3x3 = 9 shifted matmuls over a [512, 35*66] zero-padded cat buffer;
  BN+ReLU fused into one activation (scale=inv, bias=beta per partition).
"""
import sys

if "/opt/trn_rl_repo" not in sys.path:
    sys.path.insert(0, "/opt/trn_rl_repo")

import numpy as np

import concourse.bass as bass
import concourse.bacc as bacc
import concourse.mybir as mybir
import concourse.tile as tile
from concourse.bass import ds, ts
from concourse.bass_utils import run_bass_kernel_spmd

F32 = mybir.dt.float32
F32R = mybir.dt.float32r  # same bits as fp32; 1 cycle/row PE mode (vs 4 for fp32)
EPS = 1e-5
P = 128
C = 256          # channels
M = 4096         # key/value positions (64*64)
NQ = 2176        # query positions per core (34 rows * 64), host padded
NROWS = 35       # cat_pad rows (34 data + 1 zero)
WPAD = 66        # cat_pad row width (64 + 2 zero cols)
# all moving dims >= 256 so the fp32r fast path applies
ATT_BLOCKS = [(i * 256, 256) for i in range(8)] + [(2048, 128)]
QWINS = [(0, 512), (512, 512), (1024, 512), (1536, 384), (1920, 256)]
CONV_WINS = [(1, 512), (513, 512), (1025, 512), (1537, 318), (1855, 256)]

_CACHE = {}


def _wins(total, w):
    return [(i, min(w, total - i)) for i in range(0, total, w)]


def _mm(nc, out, lhsT, rhs, **kw):
    nc.tensor.matmul(out, lhsT, rhs, **kw)


def _declare_io(nc):
    t = {}
    inp = lambda name, shape, dt=F32: t.__setitem__(
        name, nc.dram_tensor(name, shape, dt, kind="ExternalInput"))
    out = lambda name, shape, dt=F32: t.__setitem__(
        name, nc.dram_tensor(name, shape, dt, kind="ExternalOutput"))
    # fp32r = same 32-bit data; matmul operands must be declared fp32r end-to-end
    inp("x1", [C, M], F32R); inp("x2", [C, M], F32R)
    inp("x1q", [C, NQ], F32R); inp("x2q", [C, NQ], F32R)
    inp("x1r", [C, NQ]); inp("x2r", [C, NQ])
    inp("maskg", [P, 17])
    inp("wq1T", [P, 2, 32], F32R); inp("wq2T", [P, 2, 32], F32R)
    inp("wk1T", [P, 2, 64], F32R); inp("wk2T", [P, 2, 64], F32R)
    inp("wv1T", [P, 2, C], F32R); inp("wv2T", [P, 2, C], F32R)
    inp("bq1", [32, 1]); inp("bq2", [32, 1])
    inp("bk1", [64, 1]); inp("bk2", [64, 1])
    inp("cinv", [P, 2]); inp("cbeta", [P, 2])
    inp("wct", [3, 3, 2 * C, C], F32R)
    inp("ident", [P, P])
    out("feat", [C, 32, 64]); out("o1", [C, 32, 64], F32R); out("o2", [C, 32, 64], F32R)
    return t


def _emit(nc, tc, t, ctx):
    big = ctx.enter_context(tc.tile_pool(name="big", bufs=3))
    kqp = ctx.enter_context(tc.tile_pool(name="kq", bufs=1))
    sing = ctx.enter_context(tc.tile_pool(name="sing", bufs=1))
    expp = ctx.enter_context(tc.tile_pool(name="expp", bufs=3))
    normp = ctx.enter_context(tc.tile_pool(name="normp", bufs=3))
    scalp = ctx.enter_context(tc.tile_pool(name="scalp", bufs=4))
    resp = ctx.enter_context(tc.tile_pool(name="resp", bufs=4))
    wcp = ctx.enter_context(tc.tile_pool(name="wcp", bufs=12))
    psA = ctx.enter_context(tc.tile_pool(name="psA", bufs=2, space="PSUM"))
    psS = ctx.enter_context(tc.tile_pool(name="psS", bufs=2, space="PSUM"))

    BIG_SHAPE_BYTES = [P, 4 * NROWS * WPAD]  # cat_pad is the largest big tile

    # ---- constants / weights to SBUF ----
    idt = sing.tile([P, P], F32)
    nc.sync.dma_start(out=idt, in_=t["ident"][:])
    wq_sb, wk_sb, wv_sb, bq_sb, bk_sb = {}, {}, {}, {}, {}
    for b in (1, 2):
        wq_sb[b] = sing.tile([P, 2, 32], F32R, tag=f"wq{b}", name=f"wq{b}")
        nc.sync.dma_start(out=wq_sb[b], in_=t[f"wq{b}T"][:])
        wk_sb[b] = sing.tile([P, 2, 64], F32R, tag=f"wk{b}", name=f"wk{b}")
        nc.sync.dma_start(out=wk_sb[b], in_=t[f"wk{b}T"][:])
        wv_sb[b] = sing.tile([P, 2, C], F32R, tag=f"wv{b}", name=f"wv{b}")
        nc.sync.dma_start(out=wv_sb[b], in_=t[f"wv{b}T"][:])
        bq_sb[b] = sing.tile([32, 1], F32, tag=f"bq{b}", name=f"bq{b}")
        nc.sync.dma_start(out=bq_sb[b], in_=t[f"bq{b}"][:])
        bk_sb[b] = sing.tile([64, 1], F32, tag=f"bk{b}", name=f"bk{b}")
        nc.sync.dma_start(out=bk_sb[b], in_=t[f"bk{b}"][:])
    cinv_sb = sing.tile([P, 2], F32, tag="cinv")
    nc.sync.dma_start(out=cinv_sb, in_=t["cinv"][:])
    cbeta_sb = sing.tile([P, 2], F32, tag="cbeta")
    nc.sync.dma_start(out=cbeta_sb, in_=t["cbeta"][:])
    maskg_sb = sing.tile([P, 17], F32, tag="maskg")
    nc.sync.dma_start(out=maskg_sb, in_=t["maskg"][:])

    # ---- load x1, x2 (two column-half DMAs so the PE can start earlier) ----
    def load_x(name):
        x_sb = big.tile(BIG_SHAPE_BYTES, F32R, tag="big")
        xv = x_sb[:, : 2 * M].rearrange("p (kc n) -> p kc n", kc=2)
        src_ap = t[name][:].rearrange("(kc p) n -> p kc n", p=P)
        for c0 in range(0, M, 1024):
            nc.sync.dma_start(out=xv[:, :, ds(c0, 1024)],
                              in_=src_ap[:, :, ds(c0, 1024)])
        return xv

    x1_sb = load_x("x1")
    x2_sb = load_x("x2")

    # ---- k projections: k_b = wk_b @ x_b + bk_b, stored [128(c pad0), 4096] ----
    k_sb = {}
    for b, x_sb in ((1, x1_sb), (2, x2_sb)):
        kp = kqp.tile([P, M], F32R, tag=f"k{b}")
        for w0, ww in _wins(M, 512):
            ps = psS.tile([P, 1024], F32, tag="sc")
            for kc in range(2):
                _mm(nc, ps[0:64, :ww], wk_sb[b][:, kc, :],
                    x_sb[:, kc, ds(w0, ww)],
                    start=(kc == 0), stop=(kc == 1))
            nc.vector.tensor_scalar_add(kp[0:64, ds(w0, ww)], ps[0:64, :ww], bk_sb[b])
        k_sb[b] = kp

    # ---- vT projections: vT_b[m, c] = x_b.T @ wv_bT (no bias), plus ones col ----
    def make_vt(x_sb, b):
        vt = big.tile(BIG_SHAPE_BYTES, F32R, tag="big")
        vtv = vt[:, : 32 * 258].rearrange("p (mi c) -> p mi c", mi=32)
        nc.vector.memset(vtv[:, :, 256:257].bitcast(F32), 1.0)
        nc.vector.memset(vtv[:, :, 257:258].bitcast(F32), 0.0)
        for mi in range(32):
            ps_full = psS.tile([P, 1024], F32, tag="sc", name="vtps")
            ps = ps_full[:, :256]
            for kc in range(2):
                _mm(nc, ps, x_sb[:, kc, ts(mi, P)], wv_sb[b][:, kc, :],
                    start=(kc == 0), stop=(kc == 1))
            nc.vector.tensor_copy(out=vtv[:, mi, 0:256], in_=ps)
        return vtv

    # ---- q projection (shared by both branches): qp [128(c pad0), 2176] ----
    qp = kqp.tile([P, NQ], F32R, tag="qp")

    def q_half(name, b, row0):
        xq = big.tile(BIG_SHAPE_BYTES, F32R, tag="big")
        xqv = xq[:, : 2 * NQ].rearrange("p (kc n) -> p kc n", kc=2)
        xq_src = t[name][:].rearrange("(kc p) n -> p kc n", p=P)
        nc.sync.dma_start(out=xqv[:, :, 0:1088], in_=xq_src[:, :, 0:1088])
        nc.sync.dma_start(out=xqv[:, :, 1088:NQ], in_=xq_src[:, :, 1088:NQ])
        for w0, ww in QWINS:
            ps = psS.tile([P, 1024], F32, tag="sc")
            for kc in range(2):
                _mm(nc, ps[0:32, :ww], wq_sb[b][:, kc, :],
                    xqv[:, kc, ds(w0, ww)],
                    start=(kc == 0), stop=(kc == 1))
            nc.vector.tensor_scalar_add(qp[row0:row0 + 32, ds(w0, ww)],
                                        ps[0:32, :ww], bq_sb[b])

    q_half("x1q", 1, 0)
    vt1 = make_vt(x1_sb, 1)
    q_half("x2q", 2, 32)
    vt2 = make_vt(x2_sb, 2)

    # ---- cat_pad buffer [128, 4, 35*66], zeroed ----
    cat = big.tile(BIG_SHAPE_BYTES, F32R, tag="big")
    catv = cat[:].rearrange("p (i f) -> p i f", i=4)
    cat_r = cat[:].rearrange("p (i r w) -> p i r w", i=4, w=WPAD)
    nc.gpsimd.memset(cat[:].bitcast(F32), 0.0)

    # ---- attention branches ----
    for b, (kp, vtv, xr_name) in enumerate(
            [(k_sb[1], vt1, "x1r"), (k_sb[2], vt2, "x2r")]):
        for n0, nw in ATT_BLOCKS:
            nsub = nw // P
            g = 1024 // nw  # m-iters per exp group (4 for nw=256, 8 for 128)
            av = psA.tile([P, 1024], F32, tag="av")

            def flush_av(pend, av=av, vtv=vtv, nw=nw, nsub=nsub):
                g0, ex = pend
                for u in range(1024 // nw):
                    pmi = g0 + u
                    for j in range(nsub):
                        _mm(nc, av[:, ds(j * 512, 258)],
                            ex[:, ds(u * nw + j * P, P)], vtv[:, pmi, :],
                            start=(pmi == 0), stop=(pmi == 31))

            pend = None
            for g0 in range(0, 32, g):
                sc = psS.tile([P, 1024], F32, tag="sc")
                for u in range(g):
                    mi = g0 + u
                    _mm(nc, sc[:, ds(u * nw, nw)],
                        kp[0:64, ts(mi, P)], qp[0:64, ds(n0, nw)],
                        start=True, stop=True)
                ex = expp.tile([P, 1024], F32R, tag="ex")
                nc.scalar.activation(ex, sc, mybir.ActivationFunctionType.Exp)
                if pend is not None:
                    flush_av(pend)
                pend = (g0, ex)
            flush_av(pend)

            # epilogue per n-chunk of 128; transposes reuse the consumed AV bank
            for j in range(nsub):
                nch = n0 // P + j
                rs = scalp.tile([P, 1], F32, tag="rs")
                nc.vector.reciprocal(rs, av[:, ds(j * 512 + 256, 1)])
                nc.vector.tensor_mul(out=rs, in0=rs,
                                     in1=maskg_sb[:, ds(nch, 1)])
                nt = normp.tile([P, 256], F32, tag="nt")
                nc.vector.tensor_scalar_mul(nt, av[:, ds(j * 512, 256)], rs)
                rt = resp.tile([P, 2, P], F32, tag="rt")
                nc.sync.dma_start(
                    out=rt,
                    in_=t[xr_name][:].rearrange("(cc p) n -> p cc n", p=P)
                    [:, :, ts(nch, P)])
                for cc in range(2):
                    tp = av[:, ds(j * 512 + cc * P, P)]
                    nc.tensor.transpose(tp, nt[:, ts(cc, P)], idt)
                    nc.vector.tensor_add(
                        out=cat_r[:, 2 * b + cc, ds(2 * nch, 2), ds(1, 64)],
                        in0=tp.rearrange("p (r w) -> p r w", w=64),
                        in1=rt[:, cc, :].rearrange("p (r w) -> p r w", w=64))

        # write out this branch's attention output (rows 1..33 = the 32 real rows)
        ov = t[f"o{b + 1}"][:].rearrange("(cc p) h w -> p cc h w", p=P)
        for cc in range(2):
            nc.sync.dma_start(out=ov[:, cc],
                              in_=cat_r[:, 2 * b + cc, ds(1, 32), ds(1, 64)])

    # ---- conv 3x3 + BN + ReLU ----
    feat = big.tile(BIG_SHAPE_BYTES, F32, tag="big")
    featv = feat[:, : 2 * 2112].rearrange("p (o f) -> p o f", o=2)
    feat_r = feat[:, : 2 * 2112].rearrange("p (o r w) -> p o r w", o=2, w=WPAD)
    for oc in range(2):
        avc1 = psA.tile([P, 1024], F32, tag="av")
        avc2 = psA.tile([P, 1024], F32, tag="av")
        last = psS.tile([P, 1024], F32, tag="sc")

        def conv_dst(wi, ww, avc1=avc1, avc2=avc2, last=last):
            if wi < 2:
                return avc1[:, ds(wi * 512, ww)]
            if wi < 4:
                return avc2[:, ds((wi - 2) * 512, ww)]
            return last[:, :ww]

        wts = {}
        for ic in range(4):
            for tap in range(9):
                wt = wcp.tile([P, P], F32R, tag="wt", name=f"wt{oc}_{ic}_{tap}")
                nc.sync.dma_start(
                    out=wt, in_=t["wct"][tap // 3, tap % 3,
                                         ts(ic, P), ts(oc, P)])
                wts[(ic, tap)] = wt
        for ic in range(4):
            for tap in range(9):
                off = (tap // 3) * WPAD + (tap % 3) - 1
                for wi, (ws, ww) in enumerate(CONV_WINS):
                    _mm(nc, conv_dst(wi, ww), wts[(ic, tap)],
                        catv[:, ic, ds(ws + off, ww)],
                        start=(ic == 0 and tap == 0),
                        stop=(ic == 3 and tap == 8))
        for wi, (ws, ww) in enumerate(CONV_WINS):
            nc.scalar.activation(featv[:, oc, ds(ws, ww)], conv_dst(wi, ww),
                                 mybir.ActivationFunctionType.Relu,
                                 bias=cbeta_sb[:, ds(oc, 1)],
                                 scale=cinv_sb[:, ds(oc, 1)])
    fv = t["feat"][:].rearrange("(cc p) h w -> p cc h w", p=P)
    for oc in range(2):
        nc.sync.dma_start(out=fv[:, oc], in_=feat_r[:, oc, :, ds(1, 64)])


def _build():
    if "nc" in _CACHE:
        return _CACHE["nc"]
    nc = bacc.Bacc(None, target_bir_lowering=False)
    t = _declare_io(nc)
    from contextlib import ExitStack
    with tile.TileContext(nc) as tc, ExitStack() as ctx:
        _emit(nc, tc, t, ctx)
    nc.finalize()
    _CACHE["nc"] = nc
    return nc


def _prep_host(inputs):
    d = {k: np.ascontiguousarray(np.asarray(v, np.float32)) for k, v in inputs.items()}
    gamma = float(d["gamma"].reshape(-1)[0])
    inv = d["bn_scale"] / np.sqrt(d["bn_var"] + EPS)
    beta = d["bn_bias"] - d["bn_mean"] * inv

    def chunked(w):  # [256, o] -> [128, 2, o]
        return np.ascontiguousarray(w.reshape(2, P, -1).transpose(1, 0, 2))

    shared = {
        "wq1T": chunked(d["wq1"].T), "wq2T": chunked(d["wq2"].T),
        "wk1T": chunked(d["wk1"].T), "wk2T": chunked(d["wk2"].T),
        "wv1T": chunked(d["wv1"].T), "wv2T": chunked(d["wv2"].T),
        "bq1": d["bq1"].reshape(32, 1).copy(), "bq2": d["bq2"].reshape(32, 1).copy(),
        "bk1": d["bk1"].reshape(64, 1).copy(), "bk2": d["bk2"].reshape(64, 1).copy(),
        "cinv": np.ascontiguousarray(inv.reshape(2, P).T),
        "cbeta": np.ascontiguousarray(beta.reshape(2, P).T),
        "wct": np.ascontiguousarray(d["w_cat"].transpose(2, 3, 1, 0)),
        "ident": np.eye(P, dtype=np.float32),
    }
    gbv = {1: gamma * d["bv1"], 2: gamma * d["bv2"]}

    in_maps = []
    for core in range(8):
        s, half = core // 2, core % 2
        h0 = 32 * half
        x1 = np.ascontiguousarray(d["input1"][s].reshape(C, M))
        x2 = np.ascontiguousarray(d["input2"][s].reshape(C, M))
        n_lo, n_hi = (h0 - 1) * 64, (h0 + 33) * 64
        lo_pad, hi_pad = max(0, -n_lo), max(0, n_hi - M)
        sl = slice(n_lo + lo_pad, n_hi - hi_pad)

        def pad_slice(x, add=None):
            o = np.zeros((C, NQ), np.float32)
            body = x[:, sl]
            if add is not None:
                body = body + add[:, None]
            o[:, lo_pad:NQ - hi_pad] = body
            return o

        maskg = np.zeros(NQ, np.float32)
        maskg[lo_pad:NQ - hi_pad] = gamma
        m = dict(shared)
        m.update({
            "x1": x1, "x2": x2,
            "x1q": pad_slice(x1), "x2q": pad_slice(x2),
            "x1r": pad_slice(x1, gbv[1]), "x2r": pad_slice(x2, gbv[2]),
            "maskg": np.ascontiguousarray(maskg.reshape(17, P).T),
        })
        in_maps.append(m)
    return in_maps


def _run_cached_pjrt(nc, in_maps):
    """run_bass_via_pjrt equivalent with the traced/jitted executable cached
    across kernel() calls (run_bass_via_pjrt rebuilds it every call)."""
    import jax
    import numpy as _np
    from jax.sharding import Mesh, PartitionSpec
    from jax.experimental.shard_map import shard_map
    from concourse import bass2jax, mybir as _mb

    n_cores = len(in_maps)
    if "pjrt" not in _CACHE:
        bass2jax.install_neuronx_cc_hook()
        in_names, out_names, out_avals, zero_shapes = [], [], [], []
        for alloc in nc.m.functions[0].allocations:
            if not isinstance(alloc, _mb.MemoryLocationSet):
                continue
            name = alloc.memorylocations[0].name
            if alloc.kind == "ExternalInput":
                if nc.partition_id_tensor is None or \
                        name != nc.partition_id_tensor.name:
                    in_names.append(name)
            elif alloc.kind == "ExternalOutput":
                out_names.append(name)
                shape = tuple(alloc.tensor_shape)
                dtype = _mb.dt.np(alloc.dtype)
                out_avals.append(jax.core.ShapedArray(shape, dtype))
                zero_shapes.append((shape, dtype))
        n_params = len(in_names)
        all_names = in_names + out_names
        pid_name = nc.partition_id_tensor.name if nc.partition_id_tensor else None
        if pid_name is not None:
            all_names = all_names + [pid_name]

        def _body(*args):
            operands = list(args)
            if pid_name is not None:
                operands.append(bass2jax.partition_id_tensor())
            outs = bass2jax._bass_exec_p.bind(
                *operands,
                out_avals=tuple(out_avals),
                in_names=tuple(all_names),
                out_names=tuple(out_names),
                lowering_input_output_aliases=(),
                sim_require_finite=True,
                sim_require_nnan=True,
                nc=nc,
            )
            return tuple(outs)

        devices = jax.devices()[:n_cores]
        mesh = Mesh(_np.asarray(devices), ("core",))
        n_outs = len(out_names)
        sharded = jax.jit(
            shard_map(_body, mesh=mesh,
                      in_specs=(PartitionSpec("core"),) * (n_params + n_outs),
                      out_specs=(PartitionSpec("core"),) * n_outs,
                      check_rep=False),
            donate_argnums=tuple(range(n_params, n_params + n_outs)),
            keep_unused=True,
        )
        _CACHE["pjrt"] = (sharded, in_names, out_names, out_avals, zero_shapes)

    sharded, in_names, out_names, out_avals, zero_shapes = _CACHE["pjrt"]
    n_cores_ax = len(in_maps)
    concat_in = [
        _np.concatenate([_np.asarray(in_maps[c][nm]) for c in range(n_cores_ax)], axis=0)
        for nm in in_names
    ]
    concat_zeros = [
        _np.zeros((n_cores_ax * s[0], *s[1:]), d) for s, d in zero_shapes
    ]
    out_arrs = sharded(*concat_in, *concat_zeros)
    return [
        {nm: _np.asarray(out_arrs[i]).reshape(n_cores_ax, *out_avals[i].shape)[c]
         for i, nm in enumerate(out_names)}
        for c in range(n_cores_ax)
    ]


def kernel(**inputs):
    nc = _build()
    in_maps = _prep_host(inputs)
    try:
        results = _run_cached_pjrt(nc, in_maps)
    except Exception:
        _CACHE.pop("pjrt", None)
        res = run_bass_kernel_spmd(nc, in_maps, core_ids=list(range(8)))
        _CACHE["last_results"] = res
        results = res.results
    feat = np.zeros((4, C, 64, 64), np.float32)
    o1 = np.zeros((4, C, 64, 64), np.float32)
    o2 = np.zeros((4, C, 64, 64), np.float32)
    for core in range(8):
        s, half = core // 2, core % 2
        r = results[core]
        feat[s, :, 32 * half:32 * half + 32] = r["feat"]
        o1[s, :, 32 * half:32 * half + 32] = r["o1"]
        o2[s, :, 32 * half:32 * half + 32] = r["o2"]
    return (feat, o1, o2)



# revision 39
# speedup vs baseline: 1.8287x; 1.8287x over previous
"""Trainium2 Bass kernel for nn_CrossAtt (dual cross-attention + 3x3 conv + BN + ReLU).

Sharding: 8 cores = (sample s in 0..3) x (h-half in 0..1). Each core computes
its 32 output rows plus a 1-row attention halo on each side (34 rows = 2176
query positions, host-zero-padded so the program is SPMD-uniform), then runs
the 3x3 conv locally. No collectives.

Device layout choices (fp8 DoubleRow edition):
- q/k/v/scores/AV matmuls all run in fp8e4 with MatmulPerfMode.DoubleRow
  (0.5 cycles per output column, contracting 2x128 per call).
- k bias is dropped: it only shifts attention logits by a per-query constant,
  which softmax cancels exactly. q bias is applied in the PSUM->fp8 cast.
- exp on ScalarE writes fp8 directly with bias=-1 (softmax-invariant shift
  that guards the fp8e4 max of 240 against exp(|s|<~5.5)).
- AV computes outT[n, c] over m-tile PAIRS; col 256 of vT accumulates the
  softmax denominator S for free.
- scores/exp/AV process n-blocks of 128 queries; PSUM plan: scores 2x2 banks,
  AV 2x1, conv 2x1 = 8 banks.
- 3x3 conv stays fp32r (feat needs the precision) and its 360 matmuls are
  pumped into the PE stream between attention blocks as their cat rows become
  ready, hiding them under the Activation-bound softmax. BN runs on DVE and
  ReLU on gpsimd so ScalarE does exp only.
- residual + gamma*bv bias are folded into the host-prepared x?r inputs.
"""
import sys

if "/opt/trn_rl_repo" not in sys.path:
    sys.path.insert(0, "/opt/trn_rl_repo")

import numpy as np

import concourse.bass as bass
import concourse.bacc as bacc
import concourse.mybir as mybir
import concourse.tile as tile
from concourse.bass import ds, ts
from concourse.bass_utils import run_bass_kernel_spmd

F32 = mybir.dt.float32
F32R = mybir.dt.float32r
F8 = mybir.dt.float8e4
BF16 = mybir.dt.bfloat16
DR = mybir.MatmulPerfMode.DoubleRow
EPS = 1e-5
P = 128
C = 256          # channels
M = 4096         # key/value positions (64*64)
NQ = 2176        # query positions per core (34 rows * 64), host padded
NBLK = 17        # n-blocks of 128 queries
NROWS = 35       # cat_pad rows (34 data + 1 zero)
WPAD = 66        # cat_pad row width (64 + 2 zero cols)
CONV_WINS = [(1, 512), (513, 512), (1025, 512), (1537, 318), (1855, 256)]
# earliest n-block (both branches) whose epilogue completes the cat rows a
# conv window reads: window w reads cat flat up to ws+ww-1+67 -> row R;
# block b completes cat rows 2b..2b+1.
CONV_READY = [4, 8, 12, 14, 16]

_CACHE = {}


def _declare_io(nc):
    t = {}
    inp = lambda name, shape, dt=F32: t.__setitem__(
        name, nc.dram_tensor(name, shape, dt, kind="ExternalInput"))
    out = lambda name, shape, dt=F32: t.__setitem__(
        name, nc.dram_tensor(name, shape, dt, kind="ExternalOutput"))
    # fp8 payloads cross the jax/PJRT boundary as uint8; bitcast on device
    U8 = mybir.dt.uint8
    inp("x1", [P, 2, M], U8); inp("x2", [P, 2, M], U8)
    inp("x1q", [P, 2, NQ], U8); inp("x2q", [P, 2, NQ], U8)
    inp("x1r", [C, NQ]); inp("x2r", [C, NQ])
    inp("maskg", [P, NBLK])
    inp("wq1", [P, 2, 32], U8); inp("wq2", [P, 2, 32], U8)
    inp("wk1", [P, 2, 64], U8); inp("wk2", [P, 2, 64], U8)
    inp("wv1", [P, 2, C], U8); inp("wv2", [P, 2, C], U8)
    inp("bq1", [32, 1]); inp("bq2", [32, 1])
    inp("cinv", [P, 2]); inp("cbeta", [P, 2])
    inp("wct", [P, 36, C], F32R)
    inp("identb", [P, P], BF16)
    out("feat", [C, 32, 64]); out("o1", [C, 32, 64], F32R); out("o2", [C, 32, 64], F32R)
    return t


def _emit(nc, tc, t, ctx):
    sing = ctx.enter_context(tc.tile_pool(name="sing", bufs=1))
    xp = ctx.enter_context(tc.tile_pool(name="xp", bufs=1))
    kqp = ctx.enter_context(tc.tile_pool(name="kq", bufs=1))
    vtp = ctx.enter_context(tc.tile_pool(name="vtp", bufs=1))
    wcp = ctx.enter_context(tc.tile_pool(name="wcp", bufs=1))
    catp = ctx.enter_context(tc.tile_pool(name="catp", bufs=1))
    featp = ctx.enter_context(tc.tile_pool(name="featp", bufs=1))
    expp = ctx.enter_context(tc.tile_pool(name="expp", bufs=3))
    ntp = ctx.enter_context(tc.tile_pool(name="ntp", bufs=3))
    rtp = ctx.enter_context(tc.tile_pool(name="rtp", bufs=4))
    scalp = ctx.enter_context(tc.tile_pool(name="scalp", bufs=4))
    psS = ctx.enter_context(tc.tile_pool(name="psS", bufs=2, space="PSUM"))
    psA = ctx.enter_context(tc.tile_pool(name="psA", bufs=2, space="PSUM"))
    psC = ctx.enter_context(tc.tile_pool(name="psC", bufs=2, space="PSUM"))

    mm = nc.tensor.matmul

    # ---- constants / weights to SBUF ----
    idt = sing.tile([P, P], BF16, tag="identb")
    nc.sync.dma_start(out=idt, in_=t["identb"][:])
    maskg_sb = sing.tile([P, NBLK], F32, tag="maskg")
    nc.sync.dma_start(out=maskg_sb, in_=t["maskg"][:])
    cinv_sb = sing.tile([P, 2], F32, tag="cinv")
    nc.sync.dma_start(out=cinv_sb, in_=t["cinv"][:])
    cbeta_sb = sing.tile([P, 2], F32, tag="cbeta")
    nc.sync.dma_start(out=cbeta_sb, in_=t["cbeta"][:])
    # exp bias: per-query-constant shift, cancels in softmax; keeps exp < 240
    negone = sing.tile([P, 1], F32, tag="negone")
    nc.vector.memset(negone, -1.0)
    U8 = mybir.dt.uint8
    wq_sb, wk_sb, wv_sb, bq_sb = {}, {}, {}, {}
    for b in (1, 2):
        wqt = sing.tile([P, 2, 32], U8, tag=f"wq{b}", name=f"wq{b}")
        nc.sync.dma_start(out=wqt, in_=t[f"wq{b}"][:])
        wq_sb[b] = wqt[:].bitcast(F8)
        wkt = sing.tile([P, 2, 64], U8, tag=f"wk{b}", name=f"wk{b}")
        nc.sync.dma_start(out=wkt, in_=t[f"wk{b}"][:])
        wk_sb[b] = wkt[:].bitcast(F8)
        wvt = sing.tile([P, 2, C], U8, tag=f"wv{b}", name=f"wv{b}")
        nc.sync.dma_start(out=wvt, in_=t[f"wv{b}"][:])
        wv_sb[b] = wvt[:].bitcast(F8)
        bq_sb[b] = sing.tile([32, 1], F32, tag=f"bq{b}", name=f"bq{b}")
        nc.sync.dma_start(out=bq_sb[b], in_=t[f"bq{b}"][:])

    # ---- x loads (fp8 bits as uint8, column-split for earlier PE start) ----
    x_sb, xq_sb = {}, {}
    for b in (1, 2):
        xs = xp.tile([P, 2, M], U8, tag=f"x{b}", name=f"x{b}")
        for c0 in range(0, M, 2048):
            nc.sync.dma_start(out=xs[:, :, ds(c0, 2048)],
                              in_=t[f"x{b}"][:][:, :, ds(c0, 2048)])
        x_sb[b] = xs[:].bitcast(F8)
        xq = xp.tile([P, 2, NQ], U8, tag=f"xq{b}", name=f"xq{b}")
        nc.sync.dma_start(out=xq, in_=t[f"x{b}q"][:])
        xq_sb[b] = xq[:].bitcast(F8)

    # ---- conv weights: one big DMA per oc-half, all 36 (ic,tap) tiles ----
    wcO = []
    for oc in range(2):
        w = wcp.tile([P, 36, P], F32R, tag=f"wc{oc}", name=f"wc{oc}")
        nc.sync.dma_start(out=w, in_=t["wct"][:][:, :, ts(oc, P)])
        wcO.append(w)

    # ---- cat_pad buffer [128, 4, 35*66]: only the pad edges need zeroing ----
    cat = catp.tile([P, 4, NROWS * WPAD], F32R, tag="cat")
    cat_r = cat[:].rearrange("p i (r w) -> p i r w", w=WPAD)
    catf = cat[:].bitcast(F32).rearrange("p i (r w) -> p i r w", w=WPAD)
    nc.gpsimd.memset(catf[:, :, :, ds(0, 1)], 0.0)    # left pad col
    nc.gpsimd.memset(catf[:, :, :, ds(65, 1)], 0.0)   # right pad col
    nc.gpsimd.memset(catf[:, :, ds(34, 1), :], 0.0)   # bottom pad row
    feat = featp.tile([P, 2, 32 * WPAD], F32, tag="feat")
    feat_r = feat[:].rearrange("p o (r w) -> p o r w", w=WPAD)

    # ---- projections (all fp8 DoubleRow), engine-split casts ----
    # q (DVE, with bias), k (DVE; bias dropped: softmax-invariant),
    # vT copies (gpsimd). Branch-1 + q up front; branch-2 k/vT pumped into
    # the early attention steps (branch 2 is staggered by 4 blocks).
    vt = {b: vtp.tile([P, 32, 258], F8, tag=f"vt{b}", name=f"vt{b}")
          for b in (1, 2)}
    for b in (1, 2):
        nc.gpsimd.memset(vt[b][:, :, ds(256, 1)], 1.0)
        nc.gpsimd.memset(vt[b][:, :, ds(257, 1)], 0.0)
    k8 = {b: kqp.tile([32, 2, M], F8, tag=f"k{b}", name=f"k{b}") for b in (1, 2)}
    q8 = kqp.tile([32, 2, NQ], F8, tag="q8")

    def emit_q(b, w0, eng="dve"):
        ww = min(512, NQ - w0)
        ps = psS.tile([P, 1024], F32, tag="sc", name="qps")
        mm(ps[0:32, :ww], wq_sb[b][:], xq_sb[b][:, :, ds(w0, ww)],
           start=True, stop=True, perf_mode=DR)
        if eng == "act":
            nc.scalar.activation(q8[:, ds(b - 1, 1), ds(w0, ww)],
                                 ps[0:32, :ww],
                                 mybir.ActivationFunctionType.Identity,
                                 bias=bq_sb[b])
        else:
            nc.vector.tensor_scalar_add(
                q8[:, ds(b - 1, 1), ds(w0, ww)], ps[0:32, :ww], bq_sb[b])

    def emit_k(b, w0, pool, tag, eng="dve"):
        ps = pool.tile([P, 512], F32, tag=tag, name="kps")
        psv = ps[0:32, :].rearrange("p (s n) -> p s n", s=2)
        for s in range(2):
            mm(psv[:, s, :], wk_sb[b][:, :, ds(32 * s, 32)],
               x_sb[b][:, :, ds(w0, 256)],
               start=True, stop=True, perf_mode=DR)
        if eng == "act":  # ScalarE is idle during the lead-in: let it cast
            nc.scalar.activation(k8[b][:, :, ds(w0, 256)], psv,
                                 mybir.ActivationFunctionType.Copy)
        else:
            nc.vector.tensor_copy(out=k8[b][:, :, ds(w0, 256)], in_=psv)

    def emit_vt(b, mi, pool, tag, eng="pool"):
        ps = pool.tile([P, 512], F32, tag=tag, name="vtps")
        psv = ps[:].rearrange("p (s n) -> p s n", s=2)
        for s in range(2):
            mm(psv[:, s, :], x_sb[b][:, :, ts(mi + s, P)], wv_sb[b][:],
               start=True, stop=True, perf_mode=DR)
        if eng == "act":  # GPSIMD cannot read PSUM; ScalarE helps in lead-in
            nc.scalar.activation(vt[b][:, ds(mi, 2), 0:256], psv,
                                 mybir.ActivationFunctionType.Copy)
        else:
            nc.vector.tensor_copy(out=vt[b][:, ds(mi, 2), 0:256], in_=psv)

    # prelude: first q windows, then k1 (w0-3 casts on DVE, rest on ScalarE,
    # which is idle until the first exp) + vt1 (Pool); remaining q windows:
    # branch 1 on ScalarE, branch 2 on DVE ahead of k1's DVE share.
    emit_q(1, 0)
    emit_q(2, 0)
    for w0 in range(512, NQ, 512):
        emit_q(2, w0, "dve")
    for i in range(16):
        emit_k(1, i * 256, psA, "av", "act" if i >= 4 else "dve")
        emit_vt(1, 2 * i, psC, "cps", "act" if i >= 8 else "dve")
    for w0 in range(512, NQ, 512):
        emit_q(1, w0, "act")
    # branch-2 projections, pumped one k-window + one vt-pair per early step
    proj2 = []
    for i in range(16):
        proj2.append(lambda i=i: emit_k(2, i * 256, psC, "cps"))
        proj2.append(lambda i=i: emit_vt(2, 2 * i, psC, "cps", "dve"))
    p2state = {"i": 0}

    def proj2_pump(budget):
        while p2state["i"] < len(proj2) and budget > 0:
            proj2[p2state["i"]]()
            p2state["i"] += 1
            budget -= 1

    # ---- conv pump: emit 3x3-conv matmuls into PE gaps as rows get ready ----
    catv = cat[:]  # [P, 4, 2310]
    conv_q = []
    for w, (ws, ww) in enumerate(CONV_WINS):
        for oc in range(2):
            conv_q.append(("start", w, oc))
            for ic in range(4):
                for tap in range(9):
                    conv_q.append(("mm", w, oc, ic, tap))
            conv_q.append(("fin", w, oc))
    cstate = {"i": 0, "ps": {}}

    def emit_feat_dma(r0, rn):
        fv = t["feat"][:].rearrange("(cc p) h w -> p cc h w", p=P)
        for cc in range(2):
            nc.sync.dma_start(out=fv[:, cc, ds(r0, rn)],
                              in_=feat_r[:, cc, ds(r0, rn), ds(1, 64)])

    def conv_pump(avail_b1, avail_b2, budget):
        # ic 0,1 read branch-1 cat rows; ic 2,3 read branch-2. Gate each mm
        # on the branch whose rows it actually needs so w-windows start as
        # soon as branch 1 gets there.
        while cstate["i"] < len(conv_q):
            step = conv_q[cstate["i"]]
            kind, w, oc = step[0], step[1], step[2]
            if kind == "mm":
                ic, tap = step[3], step[4]
                if CONV_READY[w] > (avail_b1 if ic < 2 else avail_b2):
                    break
                if budget <= 0:
                    break
                budget -= 1
                ws, ww = CONV_WINS[w]
                off = (tap // 3) * WPAD + (tap % 3) - 1
                mm(cstate["ps"][(w, oc)][:, :ww], wcO[oc][:, ic * 9 + tap, :],
                   catv[:, ic, ds(ws + off, ww)],
                   start=(ic == 0 and tap == 0), stop=(ic == 3 and tap == 8))
            elif kind == "start":
                if CONV_READY[w] > avail_b1:
                    break
                cstate["ps"][(w, oc)] = psC.tile([P, 512], F32, tag="cps", name="cps")
            else:  # fin: BN on DVE, ReLU on gpsimd, then chase with feat DMAs
                if CONV_READY[w] > avail_b2:
                    break
                ws, ww = CONV_WINS[w]
                fv = feat[:, oc, ds(ws, ww)]
                nc.vector.tensor_scalar(
                    out=fv, in0=cstate["ps"][(w, oc)][:, :ww],
                    scalar1=cinv_sb[:, ds(oc, 1)], scalar2=cbeta_sb[:, ds(oc, 1)],
                    op0=mybir.AluOpType.mult, op1=mybir.AluOpType.add)
                nc.gpsimd.tensor_scalar_max(fv, fv, 0.0)
                if (w, oc) == (2, 1):
                    emit_feat_dma(0, 16)
                elif (w, oc) == (4, 1):
                    emit_feat_dma(16, 16)
            cstate["i"] += 1

    # ---- attention blocks (128 queries each), branches interleaved ----
    xr_ap = {b: t[f"x{b}r"][:].rearrange("(cc p) n -> p cc n", p=P) for b in (1, 2)}
    ov_ap = {b: t[f"o{b}"][:].rearrange("(cc p) h w -> p cc h w", p=P) for b in (1, 2)}

    # Flat software pipeline over (branch, block, m-group) steps. Per step the
    # PE does: scores(i) [213ns], conv slice [~850ns], AV(i-1) [215ns] — which
    # matches one exp [~1.05us] on ScalarE, so the Activation engine (the
    # bottleneck) never starves. Epilogue PE work (transposes) is deferred one
    # step so its DVE input (normalize) is ready when the in-order PE reaches it.
    # branch 2 staggered 4 blocks behind branch 1 so its k/vT (pumped into the
    # early steps) are ready when its first block runs.
    STAG = 4
    order = [(1, j) for j in range(STAG)]
    for tt in range(NBLK - STAG):
        order += [(1, STAG + tt), (2, tt)]
    order += [(2, j) for j in range(NBLK - STAG, NBLK)]
    steps = [(b, j, g) for (b, j) in order for g in range(4)]
    av_t, rt_t = {}, {}

    def emit_scores(b, j, g):
        if g == 0:
            av_t[(b, j)] = psA.tile([P, 512], F32, tag="av", name="av")
            rt = rtp.tile([P, 2, P], F32, tag="rt", name="rt")
            nc.sync.dma_start(out=rt, in_=xr_ap[b][:, :, ts(j, P)])
            rt_t[(b, j)] = rt
        sc = psS.tile([P, 1024], F32, tag="sc", name="sc")
        scv = sc[:].rearrange("p (u n) -> p u n", u=8)
        for u in range(8):
            mm(scv[:, u, :], k8[b][:, :, ts(8 * g + u, P)],
               q8[:, :, ts(j, P)], start=True, stop=True, perf_mode=DR)
        ex = expp.tile([P, 8, P], F8, tag="ex", name="ex")
        nc.scalar.activation(ex.rearrange("p u n -> p (u n)"), sc,
                             mybir.ActivationFunctionType.Exp, bias=negone)
        return ex

    def emit_av(b, j, g, ex):
        av = av_t[(b, j)]
        for tt in range(4):
            pmi = 8 * g + 2 * tt
            mm(av[:, 0:258], ex[:, ds(2 * tt, 2), :],
               vt[b][:, ds(pmi, 2), 0:258],
               start=pmi == 0, stop=pmi == 30, perf_mode=DR)
        if g == 3:  # block complete: DVE part of the epilogue starts now
            rs = scalp.tile([P, 1], F32, tag="rs", name="rs")
            nc.vector.reciprocal(rs, av[:, ds(256, 1)])
            nc.vector.tensor_mul(out=rs, in0=rs, in1=maskg_sb[:, ds(j, 1)])
            nt = ntp.tile([P, 256], BF16, tag="nt", name="nt")
            nc.vector.tensor_scalar_mul(nt, av[:, 0:256], rs)
            return nt
        return None

    def emit_epilogue_pe(b, j, nt):
        av, rt = av_t.pop((b, j)), rt_t.pop((b, j))
        for cc in range(2):
            tp = av[:, ds(320 + 64 * cc, 64)].bitcast(BF16)
            nc.tensor.transpose(tp, nt[:, ts(cc, P)], idt)
            nc.vector.tensor_add(
                out=cat_r[:, 2 * (b - 1) + cc, ds(2 * j, 2), ds(1, 64)],
                in0=tp.rearrange("p (r w) -> p r w", w=64),
                in1=rt[:, cc, :].rearrange("p (r w) -> p r w", w=64))
        if j == 8:   # cat rows 1..17 complete for this branch
            for cc in range(2):
                nc.sync.dma_start(out=ov_ap[b][:, cc, 0:16],
                                  in_=cat_r[:, 2 * (b - 1) + cc, ds(1, 16), ds(1, 64)])
        elif j == NBLK - 1:
            for cc in range(2):
                nc.sync.dma_start(out=ov_ap[b][:, cc, 16:32],
                                  in_=cat_r[:, 2 * (b - 1) + cc, ds(17, 16), ds(1, 64)])

    pend = None          # (b, j, g, ex) awaiting AV
    pend_epi = None      # (b, j, nt) awaiting PE transposes
    done = {1: -1, 2: -1}
    for (b, j, g) in steps:
        ex = emit_scores(b, j, g)
        proj2_pump(4)
        if pend_epi is not None:
            pb, pj, nt = pend_epi
            emit_epilogue_pe(pb, pj, nt)
            done[pb] = pj
            pend_epi = None
        conv_pump(done[1], done[2], 3)
        if pend is not None:
            pb, pj, pg, pex = pend
            nt = emit_av(pb, pj, pg, pex)
            if nt is not None:
                pend_epi = (pb, pj, nt)
        conv_pump(done[1], done[2], 2)
        pend = (b, j, g, ex)
    nt = emit_av(*pend[:3], pend[3])
    emit_epilogue_pe(pend[0], pend[1], nt)
    conv_pump(NBLK - 1, NBLK - 1, 1 << 30)


def _build():
    if "nc" in _CACHE:
        return _CACHE["nc"]
    nc = bacc.Bacc(None, target_bir_lowering=False)
    t = _declare_io(nc)
    from contextlib import ExitStack
    with tile.TileContext(nc) as tc, ExitStack() as ctx:
        _emit(nc, tc, t, ctx)
    nc.finalize()
    _CACHE["nc"] = nc
    return nc


def _prep_host(inputs):
    import ml_dtypes
    F8NP = ml_dtypes.float8_e4m3
    BF16NP = ml_dtypes.bfloat16
    d = {k: np.ascontiguousarray(np.asarray(v, np.float32)) for k, v in inputs.items()}
    gamma = float(d["gamma"].reshape(-1)[0])
    inv = d["bn_scale"] / np.sqrt(d["bn_var"] + EPS)
    beta = d["bn_bias"] - d["bn_mean"] * inv

    def chunk8(w):  # [o, 256] weights -> [128, 2, o] fp8 bits as uint8
        return np.ascontiguousarray(
            w.T.reshape(2, P, -1).transpose(1, 0, 2).astype(F8NP)).view(np.uint8)

    shared = {
        "wq1": chunk8(d["wq1"]), "wq2": chunk8(d["wq2"]),
        "wk1": chunk8(d["wk1"]), "wk2": chunk8(d["wk2"]),
        "wv1": chunk8(d["wv1"]), "wv2": chunk8(d["wv2"]),
        "bq1": d["bq1"].reshape(32, 1).copy(), "bq2": d["bq2"].reshape(32, 1).copy(),
        "cinv": np.ascontiguousarray(inv.reshape(2, P).T),
        "cbeta": np.ascontiguousarray(beta.reshape(2, P).T),
        # [p, ic*9 + t1*3 + t2, o] = w_cat[o, ic*128+p, t1, t2]
        "wct": np.ascontiguousarray(
            d["w_cat"].transpose(1, 2, 3, 0).reshape(4, P, 9, C)
            .transpose(1, 0, 2, 3).reshape(P, 36, C)),
        "identb": np.eye(P, dtype=BF16NP),
    }
    gbv = {1: gamma * d["bv1"], 2: gamma * d["bv2"]}

    def chunkx8(x):  # [256, n] -> [128, 2, n] fp8 bits as uint8
        return np.ascontiguousarray(
            x.reshape(2, P, -1).transpose(1, 0, 2).astype(F8NP)).view(np.uint8)

    in_maps = []
    for core in range(8):
        s, half = core // 2, core % 2
        h0 = 32 * half
        x1 = np.ascontiguousarray(d["input1"][s].reshape(C, M))
        x2 = np.ascontiguousarray(d["input2"][s].reshape(C, M))
        n_lo, n_hi = (h0 - 1) * 64, (h0 + 33) * 64
        lo_pad, hi_pad = max(0, -n_lo), max(0, n_hi - M)
        sl = slice(n_lo + lo_pad, n_hi - hi_pad)

        def pad_slice(x, add=None):
            o = np.zeros((C, NQ), np.float32)
            body = x[:, sl]
            if add is not None:
                body = body + add[:, None]
            o[:, lo_pad:NQ - hi_pad] = body
            return o

        maskg = np.zeros(NQ, np.float32)
        maskg[lo_pad:NQ - hi_pad] = gamma
        m = dict(shared)
        m.update({
            "x1": chunkx8(x1), "x2": chunkx8(x2),
            "x1q": chunkx8(pad_slice(x1)), "x2q": chunkx8(pad_slice(x2)),
            "x1r": pad_slice(x1, gbv[1]), "x2r": pad_slice(x2, gbv[2]),
            "maskg": np.ascontiguousarray(maskg.reshape(NBLK, P).T),
        })
        in_maps.append(m)
    return in_maps


def _run_cached_pjrt(nc, in_maps):
    """run_bass_via_pjrt equivalent with the traced/jitted executable cached
    across kernel() calls (run_bass_via_pjrt rebuilds it every call)."""
    import jax
    import numpy as _np
    from jax.sharding import Mesh, PartitionSpec
    from jax.experimental.shard_map import shard_map
    from concourse import bass2jax, mybir as _mb

    n_cores = len(in_maps)
    if "pjrt" not in _CACHE:
        bass2jax.install_neuronx_cc_hook()
        in_names, out_names, out_avals, zero_shapes = [], [], [], []
        for alloc in nc.m.functions[0].allocations:
            if not isinstance(alloc, _mb.MemoryLocationSet):
                continue
            name = alloc.memorylocations[0].name
            if alloc.kind == "ExternalInput":
                if nc.partition_id_tensor is None or \
                        name != nc.partition_id_tensor.name:
                    in_names.append(name)
            elif alloc.kind == "ExternalOutput":
                out_names.append(name)
                shape = tuple(alloc.tensor_shape)
                dtype = _mb.dt.np(alloc.dtype)
                out_avals.append(jax.core.ShapedArray(shape, dtype))
                zero_shapes.append((shape, dtype))
        n_params = len(in_names)
        all_names = in_names + out_names
        pid_name = nc.partition_id_tensor.name if nc.partition_id_tensor else None
        if pid_name is not None:
            all_names = all_names + [pid_name]

        def _body(*args):
            operands = list(args)
            if pid_name is not None:
                operands.append(bass2jax.partition_id_tensor())
            outs = bass2jax._bass_exec_p.bind(
                *operands,
                out_avals=tuple(out_avals),
                in_names=tuple(all_names),
                out_names=tuple(out_names),
                lowering_input_output_aliases=(),
                sim_require_finite=True,
                sim_require_nnan=True,
                nc=nc,
            )
            return tuple(outs)

        devices = jax.devices()[:n_cores]
        mesh = Mesh(_np.asarray(devices), ("core",))
        n_outs = len(out_names)
        sharded = jax.jit(
            shard_map(_body, mesh=mesh,
                      in_specs=(PartitionSpec("core"),) * (n_params + n_outs),
                      out_specs=(PartitionSpec("core"),) * n_outs,
                      check_rep=False),
            donate_argnums=tuple(range(n_params, n_params + n_outs)),
            keep_unused=True,
        )
        _CACHE["pjrt"] = (sharded, in_names, out_names, out_avals, zero_shapes)

    sharded, in_names, out_names, out_avals, zero_shapes = _CACHE["pjrt"]
    n_cores_ax = len(in_maps)
    concat_in = [
        _np.concatenate([_np.asarray(in_maps[c][nm]) for c in range(n_cores_ax)], axis=0)
        for nm in in_names
    ]
    concat_zeros = [
        _np.zeros((n_cores_ax * s[0], *s[1:]), d) for s, d in zero_shapes
    ]
    out_arrs = sharded(*concat_in, *concat_zeros)
    return [
        {nm: _np.asarray(out_arrs[i]).reshape(n_cores_ax, *out_avals[i].shape)[c]
         for i, nm in enumerate(out_names)}
        for c in range(n_cores_ax)
    ]


def kernel(**inputs):
    nc = _build()
    in_maps = _prep_host(inputs)
    try:
        results = _run_cached_pjrt(nc, in_maps)
    except Exception:
        _CACHE.pop("pjrt", None)
        res = run_bass_kernel_spmd(nc, in_maps, core_ids=list(range(8)))
        _CACHE["last_results"] = res
        results = res.results
    feat = np.zeros((4, C, 64, 64), np.float32)
    o1 = np.zeros((4, C, 64, 64), np.float32)
    o2 = np.zeros((4, C, 64, 64), np.float32)
    for core in range(8):
        s, half = core // 2, core % 2
        r = results[core]
        feat[s, :, 32 * half:32 * half + 32] = r["feat"]
        o1[s, :, 32 * half:32 * half + 32] = r["o1"]
        o2[s, :, 32 * half:32 * half + 32] = r["o2"]
    return (feat, o1, o2)
